# revision 32
# baseline (speedup 1.0000x reference)
"""Trainium2 Bass kernel for multi-head attention (Llama-style, GQA 32q/8kv,
RoPE, non-causal softmax as in the source module) distributed over 8
NeuronCores.

Distribution (tensor-parallel by heads, then token-parallel o_proj):
  phase 1: each core computes qT/kT (RoPE'd, transposed [hd, tok] layout) and
           v [tok, hd] for its 4 q-heads / 1 kv-head, from a shared x^T input.
  phase 2: flash-style attention per (batch, head): scoresT [s, t] -> exp on
           ScalarE (scale folded in) -> P@V + ones-matmul denominator in PSUM
           -> normalize -> oT [hd, tok] bf16.
  phase 3: 4MB AllToAll: head-blocks <-> token-blocks, so each core ends up
           with ALL heads for its 512-token slice.
  phase 4: o_proj with the full wo on each core's token slice; output is a
           disjoint [512, 4096] fp32 slice, host concatenates.

All matmuls run in bf16 with fp32 PSUM accumulation (fp32 matmul is 1/4 rate
on TRN2). The RoPE even/odd pairing is turned into contiguous 64-partition
blocks by permuting wq/wk columns on the host (scores are invariant to any
head-dim permutation applied to both q and k).
"""

import math
from contextlib import ExitStack
from dataclasses import dataclass

import numpy as np
import ml_dtypes

import concourse.bass as bass
import concourse.bass_isa as bass_isa
import concourse.mybir as mybir
import concourse.tile as tile
from concourse import bacc
from concourse.masks import make_identity

BF16 = mybir.dt.bfloat16
F32 = mybir.dt.float32
AF = mybir.ActivationFunctionType


@dataclass(frozen=True)
class Cfg:
    B: int = 2
    T: int = 2048          # sequence length (per batch)
    D: int = 4096          # model dim
    H: int = 32            # query heads
    HKV: int = 8           # kv heads
    HD: int = 128          # head dim (must be 128)
    NC: int = 8            # cores
    PCH: int = 256         # token chunk for projections (phase 1)
    TCH: int = 512         # token chunk for attention (phase 2)
    SGRP: int = 2          # s-tiles (128) per scores psum group (2 banks)

    @property
    def TOK(self):
        return self.B * self.T

    @property
    def HQC(self):
        return self.H // self.NC    # q heads per core

    @property
    def KD(self):
        return self.D // 128        # contraction tiles over D

    @property
    def NPCH(self):
        return self.TOK // self.PCH  # projection chunks overall

    @property
    def NTCHB(self):
        return self.T // self.TCH    # attention chunks per batch

    @property
    def NST(self):
        return self.T // 128         # s-tiles per batch

    @property
    def TBLK(self):
        return self.TOK // self.NC   # tokens per core after exchange


FULL = Cfg()


def build_nc(cfg: Cfg = FULL, collective: bool = True) -> bass.Bass:
    """Build the SPMD per-core Bass program (identical on all cores).

    collective=False replaces the AllToAll with an identity (reads a2a_in in
    phase 3) — wrong results, used only for single-core TimelineSim profiling.
    """
    B, T, D, HD, NC = cfg.B, cfg.T, cfg.D, cfg.HD, cfg.NC
    HQC, KD, PCH, TCH = cfg.HQC, cfg.KD, cfg.PCH, cfg.TCH
    TOK, TBLK = cfg.TOK, cfg.TBLK
    assert HD == 128 and D % 128 == 0 and cfg.T % TCH == 0 and TOK % NC == 0
    assert cfg.T % PCH == 0 and TBLK % 128 == 0

    nc = bacc.Bacc(
        "TRN2",
        target_bir_lowering=False,
        debug=False,
        num_devices=NC,
    )

    # ---- kernel I/O (per core) ----
    xT = nc.declare_dram_parameter("xT", [D, TOK], BF16, isOutput=False)
    wq = nc.declare_dram_parameter("wq", [D, HQC * HD], BF16, isOutput=False)
    wk = nc.declare_dram_parameter("wk", [D, HD], BF16, isOutput=False)
    wv = nc.declare_dram_parameter("wv", [D, HD], BF16, isOutput=False)
    wo = nc.declare_dram_parameter("wo", [NC * HQC * HD, D], BF16, isOutput=False)
    cosT = nc.declare_dram_parameter("cosT", [64, T], F32, isOutput=False)
    sinT = nc.declare_dram_parameter("sinT", [64, T], F32, isOutput=False)
    out = nc.declare_dram_parameter("out", [TBLK, D], F32, isOutput=True)

    # tiled DRAM views: [p, ko, free]
    xT_v = xT.rearrange("(ko p) t -> p ko t", p=128)
    wq_v = wq.rearrange("(ko p) m -> p ko m", p=128)
    wk_v = wk.rearrange("(ko p) m -> p ko m", p=128)
    wv_v = wv.rearrange("(ko p) m -> p ko m", p=128)
    wo_v = wo.rearrange("(ko p) d -> p ko d", p=128)

    scale = 1.0 / math.sqrt(HD)

    with ExitStack() as ctx:
        tc = ctx.enter_context(tile.TileContext(nc))

        per = ctx.enter_context(tc.tile_pool(name="per", bufs=1))
        dram = ctx.enter_context(tc.tile_pool(name="dram", bufs=1, space="DRAM"))
        ones_sb = per.tile([128, 128], F32)
        nc.any.memset(ones_sb[:], 1.0)
        ident_sb = per.tile([128, 128], BF16)
        make_identity(nc, ident_sb[:])
        # one exchange buffer pair per local head: each head's AllToAll is
        # issued as soon as that head's attention finishes, hiding the
        # transfers behind the remaining heads' compute
        a2a_in = [dram.tile([NC, HD, TBLK], BF16, name=f"a2ai{h}")
                  for h in range(HQC)]
        a2a_out = [dram.tile([NC, HD, TBLK], BF16, name=f"a2ao{h}")
                   for h in range(HQC)]

        # pools alive through phases 1+2
        with ExitStack() as ctx12:
            qkv = ctx12.enter_context(tc.tile_pool(name="qkv", bufs=1))
            qT_sb = [qkv.tile([128, TOK], BF16, name=f"qT{h}") for h in range(HQC)]
            kT_sb = qkv.tile([128, TOK], BF16)
            v_sb = qkv.tile([128, TOK // 128, HD], BF16)
            oT_sb = [qkv.tile([128, TOK], BF16, name=f"oT{h}") for h in range(HQC)]

            # ---------- phase 1: QKV projections + RoPE ----------
            with tc.tile_pool(name="w1", bufs=1) as w1, \
                 tc.tile_pool(name="p1", bufs=2) as p1, \
                 tc.tile_pool(name="p1ps", bufs=2, space="PSUM") as p1ps, \
                 tc.tile_pool(name="rope", bufs=3) as rope_pool:

                wq_sb = w1.tile([128, KD, HQC * HD], BF16)
                wk_sb = w1.tile([128, KD, HD], BF16)
                wv_sb = w1.tile([128, KD, HD], BF16)
                cos_sb = w1.tile([64, T], F32)
                sin_sb = w1.tile([64, T], F32)
                # first Q matmuls need only wq[kg0] + xt0[kg0]: issue those
                # first so PE starts ~10us in, stream the rest behind them.
                xt0 = p1.tile([128, KD, PCH], BF16, tag="xt")
                for kg in range(0, KD, 8):
                    kge = min(kg + 8, KD)
                    nc.sync.dma_start(wq_sb[:, kg:kge, :], wq_v[:, kg:kge, :])
                    nc.sync.dma_start(xt0[:, kg:kge, :],
                                      xT_v[:, kg:kge, 0:PCH])
                nc.sync.dma_start(cos_sb[:], cosT[:])
                nc.sync.dma_start(sin_sb[:], sinT[:])
                nc.sync.dma_start(wk_sb[:], wk_v)
                nc.sync.dma_start(wv_sb[:], wv_v)

                def rope_apply(dst, psum, pos0):
                    """psum [128, PCH] fp32 (evens on parts 0:64, odds 64:128)
                    -> dst [128, PCH] bf16 slice, RoPE'd."""
                    c = cos_sb[:, pos0:pos0 + PCH]
                    s = sin_sb[:, pos0:pos0 + PCH]
                    qe = psum[0:64, :]
                    qo = psum[64:128, :]
                    t0 = rope_pool.tile([64, PCH], F32, tag="ropetmp0")
                    t1 = rope_pool.tile([64, PCH], F32, tag="ropetmp1")
                    nc.vector.tensor_mul(t0[:], qe, c)
                    nc.vector.tensor_mul(t1[:], qo, s)
                    nc.vector.tensor_sub(dst[0:64, :], t0[:], t1[:])
                    t2 = rope_pool.tile([64, PCH], F32, tag="ropetmp0")
                    t3 = rope_pool.tile([64, PCH], F32, tag="ropetmp1")
                    nc.vector.tensor_mul(t2[:], qe, s)
                    nc.vector.tensor_mul(t3[:], qo, c)
                    nc.vector.tensor_add(dst[64:128, :], t2[:], t3[:])

                for tch in range(cfg.NPCH):
                    t0g = tch * PCH                  # global token start
                    pos0 = t0g % T                   # position within batch
                    if tch == 0:
                        xt = xt0
                    else:
                        xt = p1.tile([128, KD, PCH], BF16, tag="xt")
                        for kg in range(0, KD, 8):
                            kge = min(kg + 8, KD)
                            nc.sync.dma_start(xt[:, kg:kge, :],
                                              xT_v[:, kg:kge, t0g:t0g + PCH])

                    # Q: per head, psum [128, PCH]
                    for h in range(HQC):
                        pq = p1ps.tile([128, PCH], F32, tag="pq")
                        for k in range(KD):
                            nc.tensor.matmul(
                                pq[:],
                                lhsT=wq_sb[:, k, h * HD:(h + 1) * HD],
                                rhs=xt[:, k, :],
                                start=(k == 0), stop=(k == KD - 1),
                            )
                        rope_apply(qT_sb[h][:, t0g:t0g + PCH], pq, pos0)

                    # K
                    pk = p1ps.tile([128, PCH], F32, tag="pq")
                    for k in range(KD):
                        nc.tensor.matmul(
                            pk[:], lhsT=wk_sb[:, k, :], rhs=xt[:, k, :],
                            start=(k == 0), stop=(k == KD - 1),
                        )
                    rope_apply(kT_sb[:, t0g:t0g + PCH], pk, pos0)

                    # V: project transposed (vT [hd, t], N=PCH matmuls like K)
                    # then PE-transpose 128x128 blocks into natural [s, hd].
                    pvT = p1ps.tile([128, PCH], F32, tag="pq")
                    for k in range(KD):
                        nc.tensor.matmul(
                            pvT[:], lhsT=wv_sb[:, k, :], rhs=xt[:, k, :],
                            start=(k == 0), stop=(k == KD - 1),
                        )
                    vT_sb = rope_pool.tile([128, PCH], BF16, tag="vT")
                    nc.vector.tensor_copy(vT_sb[:], pvT[:])
                    for st in range(PCH // 128):
                        pv = p1ps.tile([128, HD], BF16, tag="pv")
                        nc.tensor.transpose(
                            pv[:], vT_sb[:, st * 128:(st + 1) * 128], ident_sb[:])
                        nc.vector.tensor_copy(v_sb[:, t0g // 128 + st, :], pv[:])

            # phase-3 pools created now (after phase-1 pools released) so the
            # stack allocator reuses phase-1 SBUF and the first wo chunk's DMA
            # can be hoisted into the phase-2 window by the scheduler.
            KO = (NC * HQC * HD) // 128   # = H*HD/128 contraction tiles
            p3 = ctx12.enter_context(tc.tile_pool(name="p3", bufs=2))
            p3o = ctx12.enter_context(tc.tile_pool(name="p3o", bufs=3))

            # ---------- phase 2: attention ----------
            SGRP = cfg.SGRP
            assert SGRP == 2
            NSG = cfg.NST // SGRP     # s-groups per batch
            with tc.tile_pool(name="p2e", bufs=4) as p2e, \
                 tc.tile_pool(name="p2sb", bufs=2) as p2sb, \
                 tc.tile_pool(name="p2ps", bufs=3, space="PSUM") as p2ps, \
                 tc.tile_pool(name="p2po", bufs=2, space="PSUM") as p2po:

                for h in range(HQC):
                    for b in range(B):
                        for tcb in range(cfg.NTCHB):
                            tg = b * T + tcb * TCH      # global token start
                            po = p2po.tile([128, TCH], F32, tag="po")
                            # per-group partial denominators (DVE, tree reduce)
                            tmps = []
                            for sg in range(NSG):
                                ps = p2ps.tile([128, SGRP * TCH], F32, tag="ps")
                                for j in range(SGRP):
                                    sidx = sg * SGRP + j       # s-tile within batch
                                    sb0 = b * T + sidx * 128   # global token of s-tile
                                    nc.tensor.matmul(
                                        ps[:, j * TCH:(j + 1) * TCH],
                                        lhsT=kT_sb[:, sb0:sb0 + 128],
                                        rhs=qT_sb[h][:, tg:tg + TCH],
                                        start=True, stop=True,
                                    )
                                e = p2e.tile([128, SGRP * TCH], BF16, tag="e")
                                nc.scalar.activation(e[:], ps[:], AF.Exp, scale=scale)
                                for j in range(SGRP):
                                    sidx = sg * SGRP + j
                                    first = (sg == 0 and j == 0)
                                    last = (sg == NSG - 1 and j == SGRP - 1)
                                    nc.tensor.matmul(
                                        po[:],
                                        lhsT=v_sb[:, (b * T) // 128 + sidx, :],
                                        rhs=e[:, j * TCH:(j + 1) * TCH],
                                        start=first, stop=last,
                                    )
                                tmp = p2sb.tile([128, TCH], BF16, tag="dtmp",
                                                bufs=NSG + 1, name=f"dt{sg}")
                                nc.vector.tensor_add(
                                    tmp[:], e[:, 0:TCH], e[:, TCH:2 * TCH])
                                tmps.append(tmp)
                            # fp32 tree over the NSG partials
                            while len(tmps) > 1:
                                nxt = []
                                for i in range(0, len(tmps) - 1, 2):
                                    s = p2sb.tile([128, TCH], F32, tag="dtree",
                                                  bufs=6, name="dtr")
                                    nc.vector.tensor_add(s[:], tmps[i][:],
                                                         tmps[i + 1][:])
                                    nxt.append(s)
                                if len(tmps) % 2:
                                    nxt.append(tmps[-1])
                                tmps = nxt
                            # partition reduction on GPSIMD (off the PE queue:
                            # a PE matmul here would head-of-line-block the
                            # next iteration behind the serial DVE tree)
                            dall = p2sb.tile([128, TCH], F32, tag="dall")
                            nc.gpsimd.partition_all_reduce(
                                dall[:], tmps[0][:], channels=128,
                                reduce_op=bass_isa.ReduceOp.add)
                            rcp = p2sb.tile([128, TCH], F32, tag="rcp")
                            nc.vector.reciprocal_approx_fast(rcp[:], dall[:])
                            nc.vector.tensor_mul(
                                oT_sb[h][:, tg:tg + TCH], po[:], rcp[:])
                            # stream this token block straight into the
                            # exchange buffer (spreads the 4MB of a2a_in
                            # writes through phase 2 instead of bunching them
                            # at the collective)
                            assert TCH % TBLK == 0, "TCH must cover whole blocks"
                            for r in range(tg // TBLK, (tg + TCH) // TBLK):
                                nc.sync.dma_start(
                                    a2a_in[h][r, :, :],
                                    oT_sb[h][:, r * TBLK:(r + 1) * TBLK],
                                )
                    # head h fully computed for all tokens: exchange it now
                    if collective:
                        nc.gpsimd.collective_compute(
                            "AllToAll",
                            mybir.AluOpType.bypass,
                            replica_groups=[list(range(NC))],
                            ins=[a2a_in[h].opt()],
                            outs=[a2a_out[h].opt()],
                        )
                if not collective:
                    a2a_out = a2a_in   # identity exchange (profiling only)

            # ---------- phase 3: o_proj on own token block, full wo ----------
            # global head g = rank*HQC + h lives in a2a_out[h][rank];
            # oall_sb slot g <- strided store. Accumulate h-major so the
            # blocks that wait on the last collective come last.
            DC = 256
            NDCH = D // DC
            korder = [i * HQC + hh for hh in range(HQC) for i in range(NC)]
            with tc.tile_pool(name="p3ps", bufs=4, space="PSUM") as p3ps:
                oall_sb = p3.tile([128, KO, TBLK], BF16, bufs=1)
                for hh in range(HQC):
                    nc.sync.dma_start(
                        oall_sb[:, hh:KO:HQC, :],
                        a2a_out[hh].rearrange("r p t -> p r t"),
                    )
                for dch in range(NDCH):
                    wo_sb = p3.tile([128, KO, DC], BF16, tag="wo_sb")
                    nc.sync.dma_start(wo_sb[:],
                                      wo_v[:, :, dch * DC:(dch + 1) * DC])
                    for tt in range(TBLK // 128):
                        pso = p3ps.tile([128, DC], F32, tag="pso")
                        for ki, k in enumerate(korder):
                            nc.tensor.matmul(
                                pso[:],
                                lhsT=oall_sb[:, k, tt * 128:(tt + 1) * 128],
                                rhs=wo_sb[:, k, :],
                                start=(ki == 0), stop=(ki == KO - 1),
                            )
                        osb = p3o.tile([128, DC], F32, tag="osb")
                        nc.vector.tensor_copy(osb[:], pso[:])
                        nc.sync.dma_start(
                            out[tt * 128:(tt + 1) * 128,
                                dch * DC:(dch + 1) * DC],
                            osb[:],
                        )

    nc.compile()
    return nc


# ------------------------------------------------------------------
# host-side input prep
# ------------------------------------------------------------------

def _rope_perm(n_heads_cols: int, HD: int) -> np.ndarray:
    """Column permutation: per head, evens first then odds."""
    idx = np.arange(n_heads_cols)
    h = idx // HD
    j = idx % HD
    # new column j' in [0,64): old 2j' ; j' in [64,128): old 2(j'-64)+1
    old = np.where(j < HD // 2, 2 * j, 2 * (j - HD // 2) + 1)
    return h * HD + old


def make_in_maps(inputs: dict, cfg: Cfg = FULL):
    B, T, D, HD, NC, HQC = cfg.B, cfg.T, cfg.D, cfg.HD, cfg.NC, cfg.HQC
    bf = ml_dtypes.bfloat16

    x = np.asarray(inputs["x"], np.float32).reshape(cfg.TOK, D)
    xT = np.ascontiguousarray(x.T).astype(bf)

    wq = np.asarray(inputs["wq"], np.float32)
    wk = np.asarray(inputs["wk"], np.float32)
    wv = np.asarray(inputs["wv"], np.float32)
    wo = np.asarray(inputs["wo"], np.float32)

    permq = _rope_perm(wq.shape[1], HD)
    permk = _rope_perm(wk.shape[1], HD)
    wq_p = wq[:, permq].astype(bf)
    wk_p = wk[:, permk].astype(bf)
    wv_b = wv.astype(bf)
    wo_b = np.ascontiguousarray(wo).astype(bf)

    cos = np.asarray(inputs["freqs_cos"], np.float32)   # [T, 64]
    sin = np.asarray(inputs["freqs_sin"], np.float32)
    cosT = np.ascontiguousarray(cos.T)
    sinT = np.ascontiguousarray(sin.T)

    in_maps = []
    for c in range(NC):
        qcols = slice(c * HQC * HD, (c + 1) * HQC * HD)
        kcols = slice(c * HD, (c + 1) * HD)
        in_maps.append({
            "xT": xT,
            "wq": np.ascontiguousarray(wq_p[:, qcols]),
            "wk": np.ascontiguousarray(wk_p[:, kcols]),
            "wv": np.ascontiguousarray(wv_b[:, kcols]),
            "wo": wo_b,
            "cosT": cosT,
            "sinT": sinT,
        })
    return in_maps


_CACHE: dict = {}


def kernel(**inputs) -> np.ndarray:
    cfg = FULL
    sp = inputs.get("start_pos", 0)
    sp = int(np.asarray(sp).reshape(-1)[0]) if np.asarray(sp).size else 0
    assert sp == 0, f"kernel only supports start_pos=0, got {sp}"

    from concourse.bass_utils import run_bass_kernel_spmd

    if "nc" not in _CACHE:
        _CACHE["nc"] = build_nc(cfg)
    nc = _CACHE["nc"]

    in_maps = make_in_maps(inputs, cfg)
    res = run_bass_kernel_spmd(nc, in_maps, list(range(cfg.NC)))
    outs = [res.results[c]["out"] for c in range(cfg.NC)]
    full = np.concatenate(outs, axis=0)          # [TOK, D]
    return full.reshape(cfg.B, cfg.T, cfg.D).astype(np.float32)


if __name__ == "__main__":
    nc = build_nc()
    n = sum(len(bb.instructions) for bb in nc.m.functions[0].blocks)
    print("built", n, "instructions")


# revision 36
# speedup vs baseline: 2.0699x; 2.0699x over previous
"""Trainium2 Bass kernel for multi-head attention (Llama-style, GQA 32q/8kv,
RoPE, non-causal softmax as in the source module) distributed over 8
NeuronCores.

Distribution (tensor-parallel by heads, then token-parallel o_proj):
  phase 1: each core computes qT/kT (RoPE'd, transposed [hd, tok] layout) and
           v [tok, hd] for its 4 q-heads / 1 kv-head, from a shared x^T input.
  phase 2: flash-style attention per (batch, head): scoresT [s, t] -> exp on
           ScalarE (scale folded in) -> P@V + ones-matmul denominator in PSUM
           -> normalize -> oT [hd, tok] bf16.
  phase 3: 4MB AllToAll: head-blocks <-> token-blocks, so each core ends up
           with ALL heads for its 512-token slice.
  phase 4: o_proj with the full wo on each core's token slice; output is a
           disjoint [512, 4096] fp32 slice, host concatenates.

All matmuls run in bf16 with fp32 PSUM accumulation (fp32 matmul is 1/4 rate
on TRN2). The RoPE even/odd pairing is turned into contiguous 64-partition
blocks by permuting wq/wk columns on the host (scores are invariant to any
head-dim permutation applied to both q and k).
"""

import math
from contextlib import ExitStack
from dataclasses import dataclass

import numpy as np
import ml_dtypes

import concourse.bass as bass
import concourse.bass_isa as bass_isa
import concourse.mybir as mybir
import concourse.tile as tile
from concourse import bacc
from concourse.masks import make_identity

BF16 = mybir.dt.bfloat16
F32 = mybir.dt.float32
AF = mybir.ActivationFunctionType


@dataclass(frozen=True)
class Cfg:
    B: int = 2
    T: int = 2048          # sequence length (per batch)
    D: int = 4096          # model dim
    H: int = 32            # query heads
    HKV: int = 8           # kv heads
    HD: int = 128          # head dim (must be 128)
    NC: int = 8            # cores
    PCH: int = 256         # token chunk for projections (phase 1)
    TCH: int = 512         # token chunk for attention (phase 2)
    SGRP: int = 2          # s-tiles (128) per scores psum group (2 banks)

    @property
    def TOK(self):
        return self.B * self.T

    @property
    def HQC(self):
        return self.H // self.NC    # q heads per core

    @property
    def KD(self):
        return self.D // 128        # contraction tiles over D

    @property
    def NPCH(self):
        return self.TOK // self.PCH  # projection chunks overall

    @property
    def NTCHB(self):
        return self.T // self.TCH    # attention chunks per batch

    @property
    def NST(self):
        return self.T // 128         # s-tiles per batch

    @property
    def TBLK(self):
        return self.TOK // self.NC   # tokens per core after exchange


FULL = Cfg()


def build_nc(cfg: Cfg = FULL, collective: bool = True) -> bass.Bass:
    """Build the SPMD per-core Bass program (identical on all cores).

    collective=False replaces the AllToAll with an identity (reads a2a_in in
    phase 3) — wrong results, used only for single-core TimelineSim profiling.
    """
    B, T, D, HD, NC = cfg.B, cfg.T, cfg.D, cfg.HD, cfg.NC
    HQC, KD, PCH, TCH = cfg.HQC, cfg.KD, cfg.PCH, cfg.TCH
    TOK, TBLK = cfg.TOK, cfg.TBLK
    assert HD == 128 and D % 128 == 0 and cfg.T % TCH == 0 and TOK % NC == 0
    assert cfg.T % PCH == 0 and TBLK % 128 == 0

    nc = bacc.Bacc(
        "TRN2",
        target_bir_lowering=False,
        debug=False,
        num_devices=NC,
    )

    # ---- kernel I/O (per core) ----
    xT = nc.declare_dram_parameter("xT", [D, TOK], BF16, isOutput=False)
    wq = nc.declare_dram_parameter("wq", [D, HQC * HD], BF16, isOutput=False)
    wk = nc.declare_dram_parameter("wk", [D, HD], BF16, isOutput=False)
    wv = nc.declare_dram_parameter("wv", [D, HD], BF16, isOutput=False)
    wo = nc.declare_dram_parameter("wo", [NC * HQC * HD, D], BF16, isOutput=False)
    cosT = nc.declare_dram_parameter("cosT", [64, T], F32, isOutput=False)
    sinT = nc.declare_dram_parameter("sinT", [64, T], F32, isOutput=False)
    out = nc.declare_dram_parameter("out", [TBLK, D], F32, isOutput=True)

    # tiled DRAM views: [p, ko, free]
    xT_v = xT.rearrange("(ko p) t -> p ko t", p=128)
    wq_v = wq.rearrange("(ko p) m -> p ko m", p=128)
    wk_v = wk.rearrange("(ko p) m -> p ko m", p=128)
    wv_v = wv.rearrange("(ko p) m -> p ko m", p=128)
    wo_v = wo.rearrange("(ko p) d -> p ko d", p=128)

    scale = 1.0 / math.sqrt(HD)

    with ExitStack() as ctx:
        tc = ctx.enter_context(tile.TileContext(nc))

        per = ctx.enter_context(tc.tile_pool(name="per", bufs=1))
        dram = ctx.enter_context(tc.tile_pool(name="dram", bufs=1, space="DRAM"))
        ident_sb = per.tile([128, 128], BF16)
        make_identity(nc, ident_sb[:])
        # Collectives on this runtime have a large fixed cost (~0.4ms each,
        # measured via R-slope timing), so do ONE AllToAll for all heads
        # rather than per-head pipelined exchanges.
        a2a_in = dram.tile([NC, HQC * HD, TBLK], BF16)
        a2a_out = dram.tile([NC, HQC * HD, TBLK], BF16)

        # pools alive through phases 1+2
        with ExitStack() as ctx12:
            qkv = ctx12.enter_context(tc.tile_pool(name="qkv", bufs=1))
            qT_sb = [qkv.tile([128, TOK], BF16, name=f"qT{h}") for h in range(HQC)]
            kT_sb = qkv.tile([128, TOK], BF16)
            v_sb = qkv.tile([128, TOK // 128, HD], BF16)
            oT_sb = [qkv.tile([128, TOK], BF16, name=f"oT{h}") for h in range(HQC)]

            # ---------- phase 1: QKV projections + RoPE ----------
            with tc.tile_pool(name="w1", bufs=1) as w1, \
                 tc.tile_pool(name="p1", bufs=2) as p1, \
                 tc.tile_pool(name="p1ps", bufs=2, space="PSUM") as p1ps, \
                 tc.tile_pool(name="rope", bufs=3) as rope_pool:

                wq_sb = w1.tile([128, KD, HQC * HD], BF16)
                wk_sb = w1.tile([128, KD, HD], BF16)
                wv_sb = w1.tile([128, KD, HD], BF16)
                cos_sb = w1.tile([64, T], F32)
                sin_sb = w1.tile([64, T], F32)
                # first Q matmuls need only wq[kg0] + xt0[kg0]: issue those
                # first so PE starts ~10us in, stream the rest behind them.
                xt0 = p1.tile([128, KD, PCH], BF16, tag="xt")
                for kg in range(0, KD, 8):
                    kge = min(kg + 8, KD)
                    nc.sync.dma_start(wq_sb[:, kg:kge, :], wq_v[:, kg:kge, :])
                    nc.sync.dma_start(xt0[:, kg:kge, :],
                                      xT_v[:, kg:kge, 0:PCH])
                nc.sync.dma_start(cos_sb[:], cosT[:])
                nc.sync.dma_start(sin_sb[:], sinT[:])
                nc.sync.dma_start(wk_sb[:], wk_v)
                nc.sync.dma_start(wv_sb[:], wv_v)

                def rope_apply(dst, psum, pos0):
                    """psum [128, PCH] fp32 (evens on parts 0:64, odds 64:128)
                    -> dst [128, PCH] bf16 slice, RoPE'd."""
                    c = cos_sb[:, pos0:pos0 + PCH]
                    s = sin_sb[:, pos0:pos0 + PCH]
                    qe = psum[0:64, :]
                    qo = psum[64:128, :]
                    t0 = rope_pool.tile([64, PCH], F32, tag="ropetmp0")
                    t1 = rope_pool.tile([64, PCH], F32, tag="ropetmp1")
                    nc.vector.tensor_mul(t0[:], qe, c)
                    nc.vector.tensor_mul(t1[:], qo, s)
                    nc.vector.tensor_sub(dst[0:64, :], t0[:], t1[:])
                    t2 = rope_pool.tile([64, PCH], F32, tag="ropetmp0")
                    t3 = rope_pool.tile([64, PCH], F32, tag="ropetmp1")
                    nc.vector.tensor_mul(t2[:], qe, s)
                    nc.vector.tensor_mul(t3[:], qo, c)
                    nc.vector.tensor_add(dst[64:128, :], t2[:], t3[:])

                for tch in range(cfg.NPCH):
                    t0g = tch * PCH                  # global token start
                    pos0 = t0g % T                   # position within batch
                    if tch == 0:
                        xt = xt0
                    else:
                        xt = p1.tile([128, KD, PCH], BF16, tag="xt")
                        for kg in range(0, KD, 8):
                            kge = min(kg + 8, KD)
                            nc.sync.dma_start(xt[:, kg:kge, :],
                                              xT_v[:, kg:kge, t0g:t0g + PCH])

                    # Q: per head, psum [128, PCH]
                    for h in range(HQC):
                        pq = p1ps.tile([128, PCH], F32, tag="pq")
                        for k in range(KD):
                            nc.tensor.matmul(
                                pq[:],
                                lhsT=wq_sb[:, k, h * HD:(h + 1) * HD],
                                rhs=xt[:, k, :],
                                start=(k == 0), stop=(k == KD - 1),
                            )
                        rope_apply(qT_sb[h][:, t0g:t0g + PCH], pq, pos0)

                    # K
                    pk = p1ps.tile([128, PCH], F32, tag="pq")
                    for k in range(KD):
                        nc.tensor.matmul(
                            pk[:], lhsT=wk_sb[:, k, :], rhs=xt[:, k, :],
                            start=(k == 0), stop=(k == KD - 1),
                        )
                    rope_apply(kT_sb[:, t0g:t0g + PCH], pk, pos0)

                    # V: project transposed (vT [hd, t], N=PCH matmuls like K)
                    # then PE-transpose 128x128 blocks into natural [s, hd].
                    pvT = p1ps.tile([128, PCH], F32, tag="pq")
                    for k in range(KD):
                        nc.tensor.matmul(
                            pvT[:], lhsT=wv_sb[:, k, :], rhs=xt[:, k, :],
                            start=(k == 0), stop=(k == KD - 1),
                        )
                    vT_sb = rope_pool.tile([128, PCH], BF16, tag="vT")
                    nc.vector.tensor_copy(vT_sb[:], pvT[:])
                    for st in range(PCH // 128):
                        pv = p1ps.tile([128, HD], BF16, tag="pv")
                        nc.tensor.transpose(
                            pv[:], vT_sb[:, st * 128:(st + 1) * 128], ident_sb[:])
                        nc.vector.tensor_copy(v_sb[:, t0g // 128 + st, :], pv[:])

            # phase-3 pools created now (after phase-1 pools released) so the
            # stack allocator reuses phase-1 SBUF and the first wo chunk's DMA
            # can be hoisted into the phase-2 window by the scheduler.
            KO = (NC * HQC * HD) // 128   # = H*HD/128 contraction tiles
            p3 = ctx12.enter_context(tc.tile_pool(name="p3", bufs=2))
            p3o = ctx12.enter_context(tc.tile_pool(name="p3o", bufs=3))

            # ---------- phase 2: attention ----------
            SGRP = cfg.SGRP
            assert SGRP == 2
            NSG = cfg.NST // SGRP     # s-groups per batch
            with tc.tile_pool(name="p2e", bufs=4) as p2e, \
                 tc.tile_pool(name="p2sb", bufs=2) as p2sb, \
                 tc.tile_pool(name="p2ps", bufs=3, space="PSUM") as p2ps, \
                 tc.tile_pool(name="p2po", bufs=2, space="PSUM") as p2po:

                for h in range(HQC):
                    for b in range(B):
                        for tcb in range(cfg.NTCHB):
                            tg = b * T + tcb * TCH      # global token start
                            po = p2po.tile([128, TCH], F32, tag="po")
                            # per-group partial denominators (DVE, tree reduce)
                            tmps = []
                            for sg in range(NSG):
                                ps = p2ps.tile([128, SGRP * TCH], F32, tag="ps")
                                for j in range(SGRP):
                                    sidx = sg * SGRP + j       # s-tile within batch
                                    sb0 = b * T + sidx * 128   # global token of s-tile
                                    nc.tensor.matmul(
                                        ps[:, j * TCH:(j + 1) * TCH],
                                        lhsT=kT_sb[:, sb0:sb0 + 128],
                                        rhs=qT_sb[h][:, tg:tg + TCH],
                                        start=True, stop=True,
                                    )
                                e = p2e.tile([128, SGRP * TCH], BF16, tag="e")
                                nc.scalar.activation(e[:], ps[:], AF.Exp, scale=scale)
                                for j in range(SGRP):
                                    sidx = sg * SGRP + j
                                    first = (sg == 0 and j == 0)
                                    last = (sg == NSG - 1 and j == SGRP - 1)
                                    nc.tensor.matmul(
                                        po[:],
                                        lhsT=v_sb[:, (b * T) // 128 + sidx, :],
                                        rhs=e[:, j * TCH:(j + 1) * TCH],
                                        start=first, stop=last,
                                    )
                                tmp = p2sb.tile([128, TCH], BF16, tag="dtmp",
                                                bufs=NSG + 1, name=f"dt{sg}")
                                nc.vector.tensor_add(
                                    tmp[:], e[:, 0:TCH], e[:, TCH:2 * TCH])
                                tmps.append(tmp)
                            # fp32 tree over the NSG partials
                            while len(tmps) > 1:
                                nxt = []
                                for i in range(0, len(tmps) - 1, 2):
                                    s = p2sb.tile([128, TCH], F32, tag="dtree",
                                                  bufs=6, name="dtr")
                                    nc.vector.tensor_add(s[:], tmps[i][:],
                                                         tmps[i + 1][:])
                                    nxt.append(s)
                                if len(tmps) % 2:
                                    nxt.append(tmps[-1])
                                tmps = nxt
                            # partition reduction on GPSIMD (off the PE queue:
                            # a PE matmul here would head-of-line-block the
                            # next iteration behind the serial DVE tree)
                            dall = p2sb.tile([128, TCH], F32, tag="dall")
                            nc.gpsimd.partition_all_reduce(
                                dall[:], tmps[0][:], channels=128,
                                reduce_op=bass_isa.ReduceOp.add)
                            rcp = p2sb.tile([128, TCH], F32, tag="rcp")
                            nc.vector.reciprocal_approx_fast(rcp[:], dall[:])
                            nc.vector.tensor_mul(
                                oT_sb[h][:, tg:tg + TCH], po[:], rcp[:])
                            # stream this token block straight into the
                            # exchange buffer (spreads the 4MB of a2a_in
                            # writes through phase 2 instead of bunching them
                            # at the collective)
                            assert TCH % TBLK == 0, "TCH must cover whole blocks"
                            for r in range(tg // TBLK, (tg + TCH) // TBLK):
                                nc.sync.dma_start(
                                    a2a_in[r, h * HD:(h + 1) * HD, :],
                                    oT_sb[h][:, r * TBLK:(r + 1) * TBLK],
                                )
                if collective:
                    nc.gpsimd.collective_compute(
                        "AllToAll",
                        mybir.AluOpType.bypass,
                        replica_groups=[list(range(NC))],
                        ins=[a2a_in.opt()],
                        outs=[a2a_out.opt()],
                    )
                else:
                    a2a_out = a2a_in   # identity exchange (profiling only)

            # ---------- phase 3: o_proj on own token block, full wo ----------
            # a2a_out[r] rows h*HD.. hold rank r's local head h = global head
            # r*HQC+h, matching wo's row order.
            DC = 256
            NDCH = D // DC
            oall_v = a2a_out.rearrange("r (ko p) t -> p (r ko) t", p=128)
            with tc.tile_pool(name="p3ps", bufs=4, space="PSUM") as p3ps:
                oall_sb = p3.tile([128, KO, TBLK], BF16, bufs=1)
                nc.sync.dma_start(oall_sb[:], oall_v)
                for dch in range(NDCH):
                    wo_sb = p3.tile([128, KO, DC], BF16, tag="wo_sb")
                    nc.sync.dma_start(wo_sb[:],
                                      wo_v[:, :, dch * DC:(dch + 1) * DC])
                    for tt in range(TBLK // 128):
                        pso = p3ps.tile([128, DC], F32, tag="pso")
                        for k in range(KO):
                            nc.tensor.matmul(
                                pso[:],
                                lhsT=oall_sb[:, k, tt * 128:(tt + 1) * 128],
                                rhs=wo_sb[:, k, :],
                                start=(k == 0), stop=(k == KO - 1),
                            )
                        osb = p3o.tile([128, DC], F32, tag="osb")
                        nc.vector.tensor_copy(osb[:], pso[:])
                        nc.sync.dma_start(
                            out[tt * 128:(tt + 1) * 128,
                                dch * DC:(dch + 1) * DC],
                            osb[:],
                        )

    nc.compile()
    return nc


# ------------------------------------------------------------------
# host-side input prep
# ------------------------------------------------------------------

def _rope_perm(n_heads_cols: int, HD: int) -> np.ndarray:
    """Column permutation: per head, evens first then odds."""
    idx = np.arange(n_heads_cols)
    h = idx // HD
    j = idx % HD
    # new column j' in [0,64): old 2j' ; j' in [64,128): old 2(j'-64)+1
    old = np.where(j < HD // 2, 2 * j, 2 * (j - HD // 2) + 1)
    return h * HD + old


def make_in_maps(inputs: dict, cfg: Cfg = FULL):
    B, T, D, HD, NC, HQC = cfg.B, cfg.T, cfg.D, cfg.HD, cfg.NC, cfg.HQC
    bf = ml_dtypes.bfloat16

    x = np.asarray(inputs["x"], np.float32).reshape(cfg.TOK, D)
    xT = np.ascontiguousarray(x.T).astype(bf)

    wq = np.asarray(inputs["wq"], np.float32)
    wk = np.asarray(inputs["wk"], np.float32)
    wv = np.asarray(inputs["wv"], np.float32)
    wo = np.asarray(inputs["wo"], np.float32)

    permq = _rope_perm(wq.shape[1], HD)
    permk = _rope_perm(wk.shape[1], HD)
    wq_p = wq[:, permq].astype(bf)
    wk_p = wk[:, permk].astype(bf)
    wv_b = wv.astype(bf)
    wo_b = np.ascontiguousarray(wo).astype(bf)

    cos = np.asarray(inputs["freqs_cos"], np.float32)   # [T, 64]
    sin = np.asarray(inputs["freqs_sin"], np.float32)
    cosT = np.ascontiguousarray(cos.T)
    sinT = np.ascontiguousarray(sin.T)

    in_maps = []
    for c in range(NC):
        qcols = slice(c * HQC * HD, (c + 1) * HQC * HD)
        kcols = slice(c * HD, (c + 1) * HD)
        in_maps.append({
            "xT": xT,
            "wq": np.ascontiguousarray(wq_p[:, qcols]),
            "wk": np.ascontiguousarray(wk_p[:, kcols]),
            "wv": np.ascontiguousarray(wv_b[:, kcols]),
            "wo": wo_b,
            "cosT": cosT,
            "sinT": sinT,
        })
    return in_maps


_CACHE: dict = {}


def kernel(**inputs) -> np.ndarray:
    cfg = FULL
    sp = inputs.get("start_pos", 0)
    sp = int(np.asarray(sp).reshape(-1)[0]) if np.asarray(sp).size else 0
    assert sp == 0, f"kernel only supports start_pos=0, got {sp}"

    from concourse.bass_utils import run_bass_kernel_spmd

    if "nc" not in _CACHE:
        _CACHE["nc"] = build_nc(cfg)
    nc = _CACHE["nc"]

    in_maps = make_in_maps(inputs, cfg)
    res = run_bass_kernel_spmd(nc, in_maps, list(range(cfg.NC)))
    outs = [res.results[c]["out"] for c in range(cfg.NC)]
    full = np.concatenate(outs, axis=0)          # [TOK, D]
    return full.reshape(cfg.B, cfg.T, cfg.D).astype(np.float32)


if __name__ == "__main__":
    nc = build_nc()
    n = sum(len(bb.instructions) for bb in nc.m.functions[0].blocks)
    print("built", n, "instructions")


# revision 37
# speedup vs baseline: 2.8745x; 1.3887x over previous
"""Trainium2 Bass kernel for multi-head attention (Llama-style, GQA 32q/8kv,
RoPE, non-causal softmax as in the source module) distributed over 8
NeuronCores.

Distribution (tensor-parallel by heads, then token-parallel o_proj):
  phase 1: each core computes qT/kT (RoPE'd, transposed [hd, tok] layout) and
           v [tok, hd] for its 4 q-heads / 1 kv-head, from a shared x^T input.
  phase 2: flash-style attention per (head, batch): scoresT [s, t] -> exp on
           ScalarE (softmax scale folded into the activation) -> P@V in PSUM;
           softmax denominators accumulate on VectorE (tree over exp tiles)
           and reduce across partitions on GPSIMD, keeping the in-order PE
           queue free of stalls -> normalize -> oT [hd, tok] bf16.
  phase 3: one 4MB AllToAll: head-blocks <-> token-blocks, so each core ends
           up with ALL heads for its 512-token slice (collectives here have
           ~0.4ms fixed cost, so exactly one is issued).
  phase 4: o_proj with the full wo on each core's token slice; output is a
           disjoint [512, 4096] fp32 slice, host concatenates.

All matmuls run in bf16 with fp32 PSUM accumulation (fp32 matmul is 1/4 rate
on TRN2). The RoPE even/odd pairing is turned into contiguous 64-partition
blocks by permuting wq/wk columns on the host (scores are invariant to any
head-dim permutation applied to both q and k).
"""

import math
from contextlib import ExitStack
from dataclasses import dataclass

import numpy as np
import ml_dtypes

import concourse.bass as bass
import concourse.bass_isa as bass_isa
import concourse.mybir as mybir
import concourse.tile as tile
from concourse import bacc
from concourse.masks import make_identity

BF16 = mybir.dt.bfloat16
F32 = mybir.dt.float32
AF = mybir.ActivationFunctionType


@dataclass(frozen=True)
class Cfg:
    B: int = 2
    T: int = 2048          # sequence length (per batch)
    D: int = 4096          # model dim
    H: int = 32            # query heads
    HKV: int = 8           # kv heads
    HD: int = 128          # head dim (must be 128)
    NC: int = 8            # cores
    PCH: int = 256         # token chunk for projections (phase 1)
    TCH: int = 512         # token chunk for attention (phase 2)
    SGRP: int = 2          # s-tiles (128) per scores psum group (2 banks)

    @property
    def TOK(self):
        return self.B * self.T

    @property
    def HQC(self):
        return self.H // self.NC    # q heads per core

    @property
    def KD(self):
        return self.D // 128        # contraction tiles over D

    @property
    def NPCH(self):
        return self.TOK // self.PCH  # projection chunks overall

    @property
    def NTCHB(self):
        return self.T // self.TCH    # attention chunks per batch

    @property
    def NST(self):
        return self.T // 128         # s-tiles per batch

    @property
    def TBLK(self):
        return self.TOK // self.NC   # tokens per core after exchange


FULL = Cfg()


def build_nc(cfg: Cfg = FULL, collective: bool = True) -> bass.Bass:
    """Build the SPMD per-core Bass program (identical on all cores).

    collective=False replaces the AllToAll with an identity (reads a2a_in in
    phase 3) — wrong results, used only for single-core TimelineSim profiling.
    """
    B, T, D, HD, NC = cfg.B, cfg.T, cfg.D, cfg.HD, cfg.NC
    HQC, KD, PCH, TCH = cfg.HQC, cfg.KD, cfg.PCH, cfg.TCH
    TOK, TBLK = cfg.TOK, cfg.TBLK
    assert HD == 128 and D % 128 == 0 and cfg.T % TCH == 0 and TOK % NC == 0
    assert cfg.T % PCH == 0 and TBLK % 128 == 0

    nc = bacc.Bacc(
        "TRN2",
        target_bir_lowering=False,
        debug=False,
        num_devices=NC,
    )

    # ---- kernel I/O (per core) ----
    xT = nc.declare_dram_parameter("xT", [D, TOK], BF16, isOutput=False)
    wq = nc.declare_dram_parameter("wq", [D, HQC * HD], BF16, isOutput=False)
    wk = nc.declare_dram_parameter("wk", [D, HD], BF16, isOutput=False)
    wv = nc.declare_dram_parameter("wv", [D, HD], BF16, isOutput=False)
    wo = nc.declare_dram_parameter("wo", [NC * HQC * HD, D], BF16, isOutput=False)
    cosT = nc.declare_dram_parameter("cosT", [64, T], F32, isOutput=False)
    sinT = nc.declare_dram_parameter("sinT", [64, T], F32, isOutput=False)
    out = nc.declare_dram_parameter("out", [TBLK, D], F32, isOutput=True)

    # tiled DRAM views: [p, ko, free]
    xT_v = xT.rearrange("(ko p) t -> p ko t", p=128)
    wq_v = wq.rearrange("(ko p) m -> p ko m", p=128)
    wk_v = wk.rearrange("(ko p) m -> p ko m", p=128)
    wv_v = wv.rearrange("(ko p) m -> p ko m", p=128)
    wo_v = wo.rearrange("(ko p) d -> p ko d", p=128)

    scale = 1.0 / math.sqrt(HD)

    with ExitStack() as ctx:
        tc = ctx.enter_context(tile.TileContext(nc))

        per = ctx.enter_context(tc.tile_pool(name="per", bufs=1))
        dram = ctx.enter_context(tc.tile_pool(name="dram", bufs=1, space="DRAM"))
        ident_sb = per.tile([128, 128], BF16)
        make_identity(nc, ident_sb[:])
        # Collectives on this runtime have a large fixed cost (~0.4ms each,
        # measured via R-slope timing), so do ONE AllToAll for all heads
        # rather than per-head pipelined exchanges.
        a2a_in = dram.tile([NC, HQC * HD, TBLK], BF16)
        a2a_out = dram.tile([NC, HQC * HD, TBLK], BF16)

        # pools alive through phases 1+2
        with ExitStack() as ctx12:
            qkv = ctx12.enter_context(tc.tile_pool(name="qkv", bufs=1))
            qT_sb = [qkv.tile([128, TOK], BF16, name=f"qT{h}") for h in range(HQC)]
            kT_sb = qkv.tile([128, TOK], BF16)
            v_sb = qkv.tile([128, TOK // 128, HD], BF16)
            oT_sb = [qkv.tile([128, TOK], BF16, name=f"oT{h}") for h in range(HQC)]

            # ---------- phase 1: QKV projections + RoPE ----------
            with tc.tile_pool(name="w1", bufs=1) as w1, \
                 tc.tile_pool(name="p1", bufs=2) as p1, \
                 tc.tile_pool(name="p1ps", bufs=2, space="PSUM") as p1ps, \
                 tc.tile_pool(name="rope", bufs=3) as rope_pool:

                wq_sb = w1.tile([128, KD, HQC * HD], BF16)
                wk_sb = w1.tile([128, KD, HD], BF16)
                wv_sb = w1.tile([128, KD, HD], BF16)
                cos_sb = w1.tile([64, T], F32)
                sin_sb = w1.tile([64, T], F32)
                # first Q matmuls need only wq[kg0] + xt0[kg0]: issue those
                # first so PE starts ~10us in, stream the rest behind them.
                xt0 = p1.tile([128, KD, PCH], BF16, tag="xt")
                for kg in range(0, KD, 8):
                    kge = min(kg + 8, KD)
                    nc.sync.dma_start(wq_sb[:, kg:kge, :], wq_v[:, kg:kge, :])
                    nc.sync.dma_start(xt0[:, kg:kge, :],
                                      xT_v[:, kg:kge, 0:PCH])
                nc.sync.dma_start(cos_sb[:], cosT[:])
                nc.sync.dma_start(sin_sb[:], sinT[:])
                nc.sync.dma_start(wk_sb[:], wk_v)
                nc.sync.dma_start(wv_sb[:], wv_v)

                def rope_apply(dst, psum, pos0):
                    """psum [128, PCH] fp32 (evens on parts 0:64, odds 64:128)
                    -> dst [128, PCH] bf16 slice, RoPE'd."""
                    c = cos_sb[:, pos0:pos0 + PCH]
                    s = sin_sb[:, pos0:pos0 + PCH]
                    qe = psum[0:64, :]
                    qo = psum[64:128, :]
                    t0 = rope_pool.tile([64, PCH], F32, tag="ropetmp0")
                    t1 = rope_pool.tile([64, PCH], F32, tag="ropetmp1")
                    nc.vector.tensor_mul(t0[:], qe, c)
                    nc.vector.tensor_mul(t1[:], qo, s)
                    nc.vector.tensor_sub(dst[0:64, :], t0[:], t1[:])
                    t2 = rope_pool.tile([64, PCH], F32, tag="ropetmp0")
                    t3 = rope_pool.tile([64, PCH], F32, tag="ropetmp1")
                    nc.vector.tensor_mul(t2[:], qe, s)
                    nc.vector.tensor_mul(t3[:], qo, c)
                    nc.vector.tensor_add(dst[64:128, :], t2[:], t3[:])

                for tch in range(cfg.NPCH):
                    t0g = tch * PCH                  # global token start
                    pos0 = t0g % T                   # position within batch
                    if tch == 0:
                        xt = xt0
                    else:
                        xt = p1.tile([128, KD, PCH], BF16, tag="xt")
                        for kg in range(0, KD, 8):
                            kge = min(kg + 8, KD)
                            nc.sync.dma_start(xt[:, kg:kge, :],
                                              xT_v[:, kg:kge, t0g:t0g + PCH])

                    # Q: per head, psum [128, PCH]
                    for h in range(HQC):
                        pq = p1ps.tile([128, PCH], F32, tag="pq")
                        for k in range(KD):
                            nc.tensor.matmul(
                                pq[:],
                                lhsT=wq_sb[:, k, h * HD:(h + 1) * HD],
                                rhs=xt[:, k, :],
                                start=(k == 0), stop=(k == KD - 1),
                            )
                        rope_apply(qT_sb[h][:, t0g:t0g + PCH], pq, pos0)

                    # K
                    pk = p1ps.tile([128, PCH], F32, tag="pq")
                    for k in range(KD):
                        nc.tensor.matmul(
                            pk[:], lhsT=wk_sb[:, k, :], rhs=xt[:, k, :],
                            start=(k == 0), stop=(k == KD - 1),
                        )
                    rope_apply(kT_sb[:, t0g:t0g + PCH], pk, pos0)

                    # V: project transposed (vT [hd, t], N=PCH matmuls like K)
                    # then PE-transpose 128x128 blocks into natural [s, hd].
                    pvT = p1ps.tile([128, PCH], F32, tag="pq")
                    for k in range(KD):
                        nc.tensor.matmul(
                            pvT[:], lhsT=wv_sb[:, k, :], rhs=xt[:, k, :],
                            start=(k == 0), stop=(k == KD - 1),
                        )
                    vT_sb = rope_pool.tile([128, PCH], BF16, tag="vT")
                    nc.vector.tensor_copy(vT_sb[:], pvT[:])
                    for st in range(PCH // 128):
                        pv = p1ps.tile([128, HD], BF16, tag="pv")
                        nc.tensor.transpose(
                            pv[:], vT_sb[:, st * 128:(st + 1) * 128], ident_sb[:])
                        nc.vector.tensor_copy(v_sb[:, t0g // 128 + st, :], pv[:])

            # phase-3 pools created now (after phase-1 pools released) so the
            # stack allocator reuses phase-1 SBUF and the first wo chunk's DMA
            # can be hoisted into the phase-2 window by the scheduler.
            KO = (NC * HQC * HD) // 128   # = H*HD/128 contraction tiles
            p3 = ctx12.enter_context(tc.tile_pool(name="p3", bufs=2))
            p3o = ctx12.enter_context(tc.tile_pool(name="p3o", bufs=3))

            # ---------- phase 2: attention ----------
            SGRP = cfg.SGRP
            assert SGRP == 2
            NSG = cfg.NST // SGRP     # s-groups per batch
            with tc.tile_pool(name="p2e", bufs=4) as p2e, \
                 tc.tile_pool(name="p2sb", bufs=2) as p2sb, \
                 tc.tile_pool(name="p2ps", bufs=3, space="PSUM") as p2ps, \
                 tc.tile_pool(name="p2po", bufs=2, space="PSUM") as p2po:

                for h in range(HQC):
                    for b in range(B):
                        for tcb in range(cfg.NTCHB):
                            tg = b * T + tcb * TCH      # global token start
                            po = p2po.tile([128, TCH], F32, tag="po")
                            # per-group partial denominators (DVE, tree reduce)
                            tmps = []
                            for sg in range(NSG):
                                ps = p2ps.tile([128, SGRP * TCH], F32, tag="ps")
                                for j in range(SGRP):
                                    sidx = sg * SGRP + j       # s-tile within batch
                                    sb0 = b * T + sidx * 128   # global token of s-tile
                                    nc.tensor.matmul(
                                        ps[:, j * TCH:(j + 1) * TCH],
                                        lhsT=kT_sb[:, sb0:sb0 + 128],
                                        rhs=qT_sb[h][:, tg:tg + TCH],
                                        start=True, stop=True,
                                    )
                                e = p2e.tile([128, SGRP * TCH], BF16, tag="e")
                                nc.scalar.activation(e[:], ps[:], AF.Exp, scale=scale)
                                for j in range(SGRP):
                                    sidx = sg * SGRP + j
                                    first = (sg == 0 and j == 0)
                                    last = (sg == NSG - 1 and j == SGRP - 1)
                                    nc.tensor.matmul(
                                        po[:],
                                        lhsT=v_sb[:, (b * T) // 128 + sidx, :],
                                        rhs=e[:, j * TCH:(j + 1) * TCH],
                                        start=first, stop=last,
                                    )
                                tmp = p2sb.tile([128, TCH], BF16, tag="dtmp",
                                                bufs=NSG + 1, name=f"dt{sg}")
                                nc.vector.tensor_add(
                                    tmp[:], e[:, 0:TCH], e[:, TCH:2 * TCH])
                                tmps.append(tmp)
                            # fp32 tree over the NSG partials
                            while len(tmps) > 1:
                                nxt = []
                                for i in range(0, len(tmps) - 1, 2):
                                    s = p2sb.tile([128, TCH], F32, tag="dtree",
                                                  bufs=6, name="dtr")
                                    nc.vector.tensor_add(s[:], tmps[i][:],
                                                         tmps[i + 1][:])
                                    nxt.append(s)
                                if len(tmps) % 2:
                                    nxt.append(tmps[-1])
                                tmps = nxt
                            # partition reduction on GPSIMD (off the PE queue:
                            # a PE matmul here would head-of-line-block the
                            # next iteration behind the serial DVE tree)
                            dall = p2sb.tile([128, TCH], F32, tag="dall")
                            nc.gpsimd.partition_all_reduce(
                                dall[:], tmps[0][:], channels=128,
                                reduce_op=bass_isa.ReduceOp.add)
                            rcp = p2sb.tile([128, TCH], F32, tag="rcp")
                            nc.vector.reciprocal_approx_fast(rcp[:], dall[:])
                            nc.vector.tensor_mul(
                                oT_sb[h][:, tg:tg + TCH], po[:], rcp[:])
                            # stream this token block straight into the
                            # exchange buffer (spreads the 4MB of a2a_in
                            # writes through phase 2 instead of bunching them
                            # at the collective)
                            assert TCH % TBLK == 0, "TCH must cover whole blocks"
                            for r in range(tg // TBLK, (tg + TCH) // TBLK):
                                nc.sync.dma_start(
                                    a2a_in[r, h * HD:(h + 1) * HD, :],
                                    oT_sb[h][:, r * TBLK:(r + 1) * TBLK],
                                )
                if collective:
                    nc.gpsimd.collective_compute(
                        "AllToAll",
                        mybir.AluOpType.bypass,
                        replica_groups=[list(range(NC))],
                        ins=[a2a_in.opt()],
                        outs=[a2a_out.opt()],
                    )
                else:
                    a2a_out = a2a_in   # identity exchange (profiling only)

            # ---------- phase 3: o_proj on own token block, full wo ----------
            # a2a_out[r] rows h*HD.. hold rank r's local head h = global head
            # r*HQC+h, matching wo's row order.
            DC = 256
            NDCH = D // DC
            oall_v = a2a_out.rearrange("r (ko p) t -> p (r ko) t", p=128)
            with tc.tile_pool(name="p3ps", bufs=4, space="PSUM") as p3ps:
                oall_sb = p3.tile([128, KO, TBLK], BF16, bufs=1)
                nc.sync.dma_start(oall_sb[:], oall_v)
                for dch in range(NDCH):
                    wo_sb = p3.tile([128, KO, DC], BF16, tag="wo_sb")
                    nc.sync.dma_start(wo_sb[:],
                                      wo_v[:, :, dch * DC:(dch + 1) * DC])
                    for tt in range(TBLK // 128):
                        pso = p3ps.tile([128, DC], F32, tag="pso")
                        for k in range(KO):
                            nc.tensor.matmul(
                                pso[:],
                                lhsT=oall_sb[:, k, tt * 128:(tt + 1) * 128],
                                rhs=wo_sb[:, k, :],
                                start=(k == 0), stop=(k == KO - 1),
                            )
                        osb = p3o.tile([128, DC], F32, tag="osb")
                        nc.vector.tensor_copy(osb[:], pso[:])
                        nc.sync.dma_start(
                            out[tt * 128:(tt + 1) * 128,
                                dch * DC:(dch + 1) * DC],
                            osb[:],
                        )

    nc.compile()
    return nc


# ------------------------------------------------------------------
# host-side input prep
# ------------------------------------------------------------------

def _rope_perm(n_heads_cols: int, HD: int) -> np.ndarray:
    """Column permutation: per head, evens first then odds."""
    idx = np.arange(n_heads_cols)
    h = idx // HD
    j = idx % HD
    # new column j' in [0,64): old 2j' ; j' in [64,128): old 2(j'-64)+1
    old = np.where(j < HD // 2, 2 * j, 2 * (j - HD // 2) + 1)
    return h * HD + old


def make_in_maps(inputs: dict, cfg: Cfg = FULL):
    B, T, D, HD, NC, HQC = cfg.B, cfg.T, cfg.D, cfg.HD, cfg.NC, cfg.HQC
    bf = ml_dtypes.bfloat16

    x = np.asarray(inputs["x"], np.float32).reshape(cfg.TOK, D)
    xT = np.ascontiguousarray(x.T).astype(bf)

    wq = np.asarray(inputs["wq"], np.float32)
    wk = np.asarray(inputs["wk"], np.float32)
    wv = np.asarray(inputs["wv"], np.float32)
    wo = np.asarray(inputs["wo"], np.float32)

    permq = _rope_perm(wq.shape[1], HD)
    permk = _rope_perm(wk.shape[1], HD)
    wq_p = wq[:, permq].astype(bf)
    wk_p = wk[:, permk].astype(bf)
    wv_b = wv.astype(bf)
    wo_b = np.ascontiguousarray(wo).astype(bf)

    cos = np.asarray(inputs["freqs_cos"], np.float32)   # [T, 64]
    sin = np.asarray(inputs["freqs_sin"], np.float32)
    cosT = np.ascontiguousarray(cos.T)
    sinT = np.ascontiguousarray(sin.T)

    in_maps = []
    for c in range(NC):
        qcols = slice(c * HQC * HD, (c + 1) * HQC * HD)
        kcols = slice(c * HD, (c + 1) * HD)
        in_maps.append({
            "xT": xT,
            "wq": np.ascontiguousarray(wq_p[:, qcols]),
            "wk": np.ascontiguousarray(wk_p[:, kcols]),
            "wv": np.ascontiguousarray(wv_b[:, kcols]),
            "wo": wo_b,
            "cosT": cosT,
            "sinT": sinT,
        })
    return in_maps


_CACHE: dict = {}


def kernel(**inputs) -> np.ndarray:
    cfg = FULL
    sp = inputs.get("start_pos", 0)
    sp = int(np.asarray(sp).reshape(-1)[0]) if np.asarray(sp).size else 0
    assert sp == 0, f"kernel only supports start_pos=0, got {sp}"

    from concourse.bass_utils import run_bass_kernel_spmd

    if "nc" not in _CACHE:
        _CACHE["nc"] = build_nc(cfg)
    nc = _CACHE["nc"]

    in_maps = make_in_maps(inputs, cfg)
    res = run_bass_kernel_spmd(nc, in_maps, list(range(cfg.NC)))
    outs = [res.results[c]["out"] for c in range(cfg.NC)]
    full = np.concatenate(outs, axis=0)          # [TOK, D]
    return full.reshape(cfg.B, cfg.T, cfg.D).astype(np.float32)


if __name__ == "__main__":
    nc = build_nc()
    n = sum(len(bb.instructions) for bb in nc.m.functions[0].blocks)
    print("built", n, "instructions")
